# revision 27
# baseline (speedup 1.0000x reference)
"""Trainium2 Bass kernel for nn_EncoderOnlyBlock (4-head full-dim encoder block).

Sharding: fully data-parallel, no collectives. 8 cores = (batch b, seq-half).
Each core computes its 1024 query tokens end-to-end for all 4 heads.

Algebraic folds (host, fp32):
  scores[i,j] = (x_i Wq + bq)·(x_j Wk + bk) / 32
              = (x_i G + w~)·x_j / 32  + terms constant over j (softmax-invariant)
    with G = Wq Wk^T, w~ = Wk bq.  K projection eliminated entirely.
  head_h @ W1_h = A_h @ (x F_h),  F_h = Wv_h W1_h;  bv/b1 fold into
    cvec = b1 + sum_h bv_h W1_h which is folded into xres on host.
  Softmax normalization deferred: proj_unnorm = sum_j exp(s) V', scaled by
    rec = 1/rowsum at PSUM->SBUF accumulation ([P,1] per-partition scalar).

fp8 (e4m3, DoubleRow perf mode = 2 K-tiles per pass) for all attention
matmuls: T' = x G'8, scores = T'8 @ x8, V' = x8 @ F'8, proj = expS8 @ V'8.
Host scales G'=32G, F'=64F keep fp8 operands in the normal range; the 32
folds into the exp scale (1/1024), the 64 into the u1 residual STT (1/64).
W2/LN tail stays bf16/f32 exactly as the bf16 baseline.

Per-core math:
  x_perm = [own-half tokens; other-half tokens]            (host permute)
  V'_h  = x_perm @ F'_h                     [sj, d]  fp8 out
  T'_h  = G'_h^T @ x_perm^T[:, :1024] + w~' [e, si]  fp8 out
  S     = T'^T x^T / 1024; E = exp(S) (fp8), r = rowsum(E) (f32 accum)
  proj += (E_h @ V'_h) * (1/r_h)            [si, d]  f32 accum in SBUF
  u1    = xres' + proj/64   (xres' = x_own + cvec, host)
  LN1 -> y; u2 = y + y@W2 + b2; LN2 -> out   (bf16 matmul, f32 LN)
"""

import numpy as np
import ml_dtypes

BF = ml_dtypes.bfloat16
F8 = ml_dtypes.float8_e4m3
P = 128
D = 1024
S = 2048
SI = 1024
H = 4
ET = D // P       # 8 e/d/f 128-blocks
SJT = S // P      # 16 sj 128-blocks
SIT = SI // P     # 8 si 128-blocks
EPS = 1e-5

_CACHE = {}


def _emit(nc, tc, A, trivial_gbe):
    """Emit the per-core program. A: dict name -> dram AP."""
    from contextlib import ExitStack

    import concourse.bass as bass
    import concourse.mybir as mybir
    from concourse.masks import make_identity

    f32 = mybir.dt.float32
    bf16 = mybir.dt.bfloat16
    f8 = mybir.dt.float8e4
    Act = mybir.ActivationFunctionType
    Alu = mybir.AluOpType
    DR = mybir.MatmulPerfMode.DoubleRow

    with ExitStack() as ctx:
        consts = ctx.enter_context(tc.tile_pool(name="consts", bufs=1))
        psA = ctx.enter_context(tc.tile_pool(name="psA", bufs=3, space="PSUM"))
        psB = ctx.enter_context(tc.tile_pool(name="psB", bufs=2, space="PSUM"))

        identb = consts.tile([P, P], bf16, tag="identb")
        make_identity(nc, identb[:])
        wtr_sb = consts.tile([P, H * ET], f32, tag="wtr")
        nc.sync.dma_start(out=wtr_sb[:], in_=A["wtr"][:])
        buv_sb = consts.tile([1, D], bf16, tag="buv")
        nc.sync.dma_start(out=buv_sb[:], in_=A["buv"][:])
        ones_sb = consts.tile([1, P], bf16, tag="ones")
        nc.vector.memset(ones_sb[:], 1.0)
        eps_sb = consts.tile([P, 1], f32, tag="eps")
        nc.vector.memset(eps_sb[:], EPS)

        head_ctx = ExitStack()
        xpool = head_ctx.enter_context(tc.tile_pool(name="xp", bufs=1))
        f_pool = head_ctx.enter_context(tc.tile_pool(name="fp8", bufs=2))
        g_pool = head_ctx.enter_context(tc.tile_pool(name="gp8", bufs=2))
        v_pool = head_ctx.enter_context(tc.tile_pool(name="vp", bufs=1))
        tt_pool = head_ctx.enter_context(tc.tile_pool(name="tt", bufs=2))
        attn_pool = head_ctx.enter_context(tc.tile_pool(name="at", bufs=3))
        atT_pool = head_ctx.enter_context(tc.tile_pool(name="atT", bufs=1))
        proj_pool = head_ctx.enter_context(tc.tile_pool(name="pj", bufs=1))
        red_pool = head_ctx.enter_context(tc.tile_pool(name="red", bufs=8))
        rec_pool = head_ctx.enter_context(tc.tile_pool(name="rec", bufs=2))

        x8_sb = xpool.tile([P, ET, S], f8, tag="x8")
        for hs in range(2):
            for c in range(ET):
                nc.sync.dma_start(
                    out=x8_sb[:, c, hs * SI:(hs + 1) * SI],
                    in_=A["x8"][c * P:(c + 1) * P, hs * SI:(hs + 1) * SI],
                )

        V8 = v_pool.tile([P, H, SJT, D], f8, tag="v8")
        proj_sb = proj_pool.tile([P, SIT, D], f32, tag="proj")

        # ---- V'_h = x @ F'_h for all 2048 rows, all heads, upfront
        for h in range(H):
            f8_sb = f_pool.tile([P, ET, D], f8, tag="f8")
            for kc in range(ET):
                nc.gpsimd.dma_start(
                    out=f8_sb[:, kc, :], in_=A["f8"][h, kc * P:(kc + 1) * P, :]
                )
            for j in range(SJT):
                ps = psA.tile([P, D], f32, tag="psA")
                for nb in range(2):
                    for kp in range(ET // 2):
                        nc.tensor.matmul(
                            ps[:, nb * 512:(nb + 1) * 512],
                            lhsT=x8_sb[:, 2 * kp:2 * kp + 2, j * P:(j + 1) * P],
                            rhs=f8_sb[:, 2 * kp:2 * kp + 2, nb * 512:(nb + 1) * 512],
                            start=(kp == 0), stop=(kp == ET // 2 - 1),
                            perf_mode=DR,
                        )
                nc.vector.tensor_copy(V8[:, h, j, :], ps[:])

        for h in range(H):
            # ---- T'^T = G'^T @ x^T[:, :1024] + w~' : [e, si]
            g8_sb = g_pool.tile([P, ET, ET, P], f8, tag="g8")
            for c in range(ET):
                nc.gpsimd.dma_start(out=g8_sb[:, c], in_=A["g8"][h, c])
            tt_sb = tt_pool.tile([P, ET, SI], f8, tag="tt")
            for c in range(ET):
                ps = psA.tile([P, D], f32, tag="psA")
                for nb in range(2):
                    for kp in range(ET // 2):
                        nc.tensor.matmul(
                            ps[:, nb * 512:(nb + 1) * 512],
                            lhsT=g8_sb[:, c, 2 * kp:2 * kp + 2, :],
                            rhs=x8_sb[:, 2 * kp:2 * kp + 2, nb * 512:(nb + 1) * 512],
                            start=(kp == 0), stop=(kp == ET // 2 - 1),
                            perf_mode=DR,
                        )
                nc.scalar.activation(
                    out=tt_sb[:, c, :], in_=ps[:], func=Act.Identity,
                    bias=wtr_sb[:, h * ET + c:h * ET + c + 1],
                )

            # ---- attention: scores+exp per si-tile, transposes pipelined
            # two tiles behind, proj chains per at-quad
            rec_sb = rec_pool.tile([P, SIT], f32, tag="rec")
            attn_tiles = [None] * SIT
            at_tiles = [None] * 4

            def scores_softmax(t):
                a_t = attn_pool.tile([P, S], bf16, tag="attn")
                attn_tiles[t] = a_t
                r = red_pool.tile([P, 2], f32, tag="rsum")
                for hs in range(2):
                    ps = psA.tile([P, D], f32, tag="psA")
                    for nb in range(2):
                        for kp in range(ET // 2):
                            nc.tensor.matmul(
                                ps[:, nb * 512:(nb + 1) * 512],
                                lhsT=tt_sb[:, 2 * kp:2 * kp + 2, t * P:(t + 1) * P],
                                rhs=x8_sb[:, 2 * kp:2 * kp + 2,
                                          hs * 1024 + nb * 512:hs * 1024 + (nb + 1) * 512],
                                start=(kp == 0), stop=(kp == ET // 2 - 1),
                                perf_mode=DR,
                            )
                    nc.scalar.activation(
                        out=a_t[:, hs * 1024:(hs + 1) * 1024], in_=ps[:],
                        func=Act.Exp, scale=1.0 / 1024.0,
                        accum_out=r[:, hs:hs + 1],
                    )
                rs = red_pool.tile([P, 1], f32, tag="rs")
                nc.vector.tensor_add(rs[:], r[:, 0:1], r[:, 1:2])
                nc.vector.reciprocal(rec_sb[:, t:t + 1], rs[:])

            def transposes(t):
                q, t2 = t // 2, t % 2
                if t2 == 0:
                    at_tiles[q] = atT_pool.tile(
                        [P, SJT, 256], f8, tag="atT", name=f"at_q{q}"
                    )
                a_t = attn_tiles[t]
                for j8 in range(2):
                    pb = psB.tile([P, 1024], bf16, tag="psB")
                    for jj in range(8):
                        j = j8 * 8 + jj
                        nc.tensor.transpose(
                            pb[:, jj * P:(jj + 1) * P],
                            a_t[:, j * P:(j + 1) * P],
                            identb[:],
                        )
                    nc.vector.tensor_copy(
                        at_tiles[q][:, j8 * 8:(j8 + 1) * 8, t2 * P:(t2 + 1) * P],
                        pb[:].rearrange("p (j c) -> p j c", c=P),
                    )
                attn_tiles[t] = None

            def proj_chains(q):
                at_sb = at_tiles[q]
                for t2 in range(2):
                    t = q * 2 + t2
                    ps = psA.tile([P, D], f32, tag="psA")
                    for nb in range(2):
                        for jp in range(SJT // 2):
                            nc.tensor.matmul(
                                ps[:, nb * 512:(nb + 1) * 512],
                                lhsT=at_sb[:, 2 * jp:2 * jp + 2, t2 * P:(t2 + 1) * P],
                                rhs=V8[:, h, 2 * jp:2 * jp + 2, nb * 512:(nb + 1) * 512],
                                start=(jp == 0), stop=(jp == SJT // 2 - 1),
                                perf_mode=DR,
                            )
                    if h == 0:
                        nc.vector.tensor_scalar_mul(
                            proj_sb[:, t, :], ps[:], rec_sb[:, t:t + 1]
                        )
                    else:
                        nc.vector.scalar_tensor_tensor(
                            out=proj_sb[:, t, :], in0=ps[:], scalar=rec_sb[:, t:t + 1],
                            in1=proj_sb[:, t, :], op0=Alu.mult, op1=Alu.add,
                        )

            scores_softmax(0)
            scores_softmax(1)
            for t in range(2, SIT):
                scores_softmax(t)
                transposes(t - 2)
                if t % 2 == 1:
                    proj_chains((t - 2) // 2)
            transposes(SIT - 2)
            transposes(SIT - 1)
            proj_chains(3)

        head_ctx.close()

        # ================= LN1 -> FFN2 -> LN2, fully per-si-tile =================
        with ExitStack() as lctx:
            lnp = lctx.enter_context(tc.tile_pool(name="lnp", bufs=1))
            xr_pool = lctx.enter_context(tc.tile_pool(name="xr", bufs=3))
            u_pool = lctx.enter_context(tc.tile_pool(name="up", bufs=3))
            sq_pool = lctx.enter_context(tc.tile_pool(name="sq", bufs=2))
            ybf_pool = lctx.enter_context(tc.tile_pool(name="ybf", bufs=2))
            yt_pool = lctx.enter_context(tc.tile_pool(name="yt", bufs=3))
            w2_pool = lctx.enter_context(tc.tile_pool(name="w2", bufs=1))
            st_pool = lctx.enter_context(tc.tile_pool(name="st", bufs=8))
            ot_pool = lctx.enter_context(tc.tile_pool(name="ot", bufs=3))

            if not trivial_gbe:
                gbe_sb = lnp.tile([P, 4, D], f32, tag="gbe")
                gbe_bc = bass.AP(
                    tensor=A["gbe"].tensor, offset=A["gbe"].offset,
                    ap=[[0, P], A["gbe"].ap[0], A["gbe"].ap[1]],
                )
                nc.gpsimd.dma_start(out=gbe_sb[:], in_=gbe_bc)
            y_sb = lnp.tile([P, SIT, D], f32, tag="y")

            xr_tiles = []
            for t in range(SIT):
                xr = xr_pool.tile([P, D], f32, tag="xr", name=f"xr{t}")
                nc.sync.dma_start(out=xr[:], in_=A["xres"][t * P:(t + 1) * P, :])
                xr_tiles.append(xr)

            w28_sb = w2_pool.tile([P, ET, D], bf16, tag="w2")
            for kc in range(ET):
                nc.sync.dma_start(
                    out=w28_sb[:, kc, :], in_=A["w2"][kc * P:(kc + 1) * P, :]
                )

            def ln_stats(src, rsum):
                """-> (mu, rstd) [P,1] tiles from src [P,D] + its row-sum."""
                sq = sq_pool.tile([P, D], f32, tag="sq")
                sumsq = st_pool.tile([P, 1], f32, tag="sumsq")
                nc.scalar.activation(out=sq[:], in_=src, func=Act.Square,
                                     accum_out=sumsq[:])
                mu = st_pool.tile([P, 1], f32, tag="mu")
                nc.scalar.mul(mu[:], rsum, 1.0 / D)
                # (rsum*mu - sumsq) = -D*var;  std = sqrt(-1/D * that + eps)
                nv = st_pool.tile([P, 1], f32, tag="nv")
                nc.vector.scalar_tensor_tensor(
                    out=nv[:], in0=rsum, scalar=mu[:], in1=sumsq[:],
                    op0=Alu.mult, op1=Alu.subtract,
                )
                rstd = st_pool.tile([P, 1], f32, tag="rstd")
                nc.scalar.activation(out=rstd[:], in_=nv[:], func=Act.Sqrt,
                                     scale=-1.0 / D, bias=eps_sb[:])
                nc.vector.reciprocal(rstd[:], rstd[:])
                return mu, rstd

            # Stage A (all tiles): u1 -> LN1 -> y -> yT. Emitting every tile's
            # A-stage before any B-stage keeps the in-order DVE queue from
            # serializing tile t+1's LN1 behind tile t's z-chain result.
            yt_tiles = []
            for t in range(SIT):
                # u1 = xres + proj/64, with row-sum accumulated in the same pass
                u1 = u_pool.tile([P, D], f32, tag="u")
                rs1 = st_pool.tile([P, 1], f32, tag="rs")
                nc.vector.scalar_tensor_tensor(
                    out=u1[:], in0=proj_sb[:, t, :], scalar=1.0 / 64.0,
                    in1=xr_tiles[t][:], op0=Alu.mult, op1=Alu.add,
                    accum_out=rs1[:],
                )
                mu1, rstd1 = ln_stats(u1[:], rs1[:])
                yt_t = y_sb[:, t, :]
                nc.vector.tensor_scalar(
                    yt_t, u1[:], scalar1=mu1[:], scalar2=rstd1[:],
                    op0=Alu.subtract, op1=Alu.mult,
                )
                if not trivial_gbe:
                    nc.gpsimd.tensor_mul(yt_t, yt_t, gbe_sb[:, 0, :])
                    nc.gpsimd.tensor_add(yt_t, yt_t, gbe_sb[:, 1, :])
                yb = ybf_pool.tile([P, D], bf16, tag="ybf")
                nc.scalar.copy(yb[:], yt_t)
                # transpose this tile's 8 f-blocks -> yT columns for its z-chain
                yt_tile = yt_pool.tile([P, ET, P], bf16, tag="yt", name=f"yt{t}")
                pb = psB.tile([P, 1024], bf16, tag="psB")
                for fb in range(ET):
                    nc.tensor.transpose(
                        pb[:, fb * P:(fb + 1) * P], yb[:, fb * P:(fb + 1) * P], identb[:]
                    )
                nc.vector.tensor_copy(
                    yt_tile[:], pb[:].rearrange("p (f c) -> p f c", c=P)
                )
                yt_tiles.append(yt_tile)

            # Stage B (all tiles): z-chain -> u2 -> LN2 -> out
            for t in range(SIT):
                ps = psA.tile([P, 1024], f32, tag="psA")
                for nb in range(2):
                    for kc in range(ET):
                        nc.tensor.matmul(
                            ps[:, nb * 512:(nb + 1) * 512],
                            lhsT=yt_tiles[t][:, kc, :],
                            rhs=w28_sb[:, kc, nb * 512:(nb + 1) * 512],
                            start=(kc == 0), stop=False,
                        )
                    nc.tensor.matmul(
                        ps[:, nb * 512:(nb + 1) * 512],
                        lhsT=ones_sb[:, :],
                        rhs=buv_sb[:, nb * 512:(nb + 1) * 512],
                        start=False, stop=True,
                    )
                u2 = u_pool.tile([P, D], f32, tag="u")
                rs2 = st_pool.tile([P, 1], f32, tag="rs")
                nc.vector.scalar_tensor_tensor(
                    out=u2[:], in0=ps[:], scalar=1.0,
                    in1=y_sb[:, t, :], op0=Alu.mult, op1=Alu.add,
                    accum_out=rs2[:],
                )
                mu2, rstd2 = ln_stats(u2[:], rs2[:])
                ot = ot_pool.tile([P, D], f32, tag="ot")
                nc.vector.tensor_scalar(
                    ot[:], u2[:], scalar1=mu2[:], scalar2=rstd2[:],
                    op0=Alu.subtract, op1=Alu.mult,
                )
                if not trivial_gbe:
                    nc.gpsimd.tensor_mul(ot[:], ot[:], gbe_sb[:, 2, :])
                    nc.gpsimd.tensor_add(ot[:], ot[:], gbe_sb[:, 3, :])
                eng = nc.sync if t % 2 == 0 else nc.gpsimd
                eng.dma_start(out=A["out"][t * P:(t + 1) * P, :], in_=ot[:])


def _build(trivial_gbe):
    import concourse.bass as bass
    import concourse.mybir as mybir
    import concourse.tile as tile
    from concourse import bacc

    f32 = mybir.dt.float32
    bf16 = mybir.dt.bfloat16
    f8 = mybir.dt.float8e4

    nc = bacc.Bacc("TRN2", target_bir_lowering=False, debug=False, num_devices=8)
    A = {}

    def din(name, shape, dt):
        A[name] = nc.dram_tensor(name, shape, dt, kind="ExternalInput").ap()

    din("x8", [D, S], f8)
    din("xres", [SI, D], f32)
    din("g8", [H, ET, P, ET, P], f8)
    din("f8", [H, D, D], f8)
    din("wtr", [P, H * ET], f32)
    din("w2", [D, D], bf16)
    din("buv", [1, D], bf16)
    if not trivial_gbe:
        din("gbe", [4, D], f32)
    A["out"] = nc.dram_tensor("out", [SI, D], f32, kind="ExternalOutput").ap()

    with tile.TileContext(nc) as tc:
        _emit(nc, tc, A, trivial_gbe)
    nc.compile()
    return nc


def _get_nc(trivial_gbe=True):
    key = ("nc", trivial_gbe)
    if key not in _CACHE:
        _CACHE[key] = _build(trivial_gbe)
    return _CACHE[key]


def _prep_inputs(inputs):
    x = np.ascontiguousarray(inputs["embedding_matrix"], dtype=np.float32)
    Wq = np.asarray(inputs["Wq"], np.float32)
    bq = np.asarray(inputs["bq"], np.float32)
    Wv = np.asarray(inputs["Wv"], np.float32)
    bv = np.asarray(inputs["bv"], np.float32)
    Wk = np.asarray(inputs["Wk"], np.float32)
    W1 = np.asarray(inputs["W1"], np.float32)
    b1 = np.asarray(inputs["b1"], np.float32)
    W2 = np.asarray(inputs["W2"], np.float32)
    b2 = np.asarray(inputs["b2"], np.float32)
    g1 = np.asarray(inputs["g1"], np.float32)
    be1 = np.asarray(inputs["be1"], np.float32)
    g2 = np.asarray(inputs["g2"], np.float32)
    be2 = np.asarray(inputs["be2"], np.float32)

    trivial = (
        np.array_equal(g1, np.ones(D, np.float32))
        and np.array_equal(g2, np.ones(D, np.float32))
        and np.array_equal(be1, np.zeros(D, np.float32))
        and np.array_equal(be2, np.zeros(D, np.float32))
    )

    # host folds
    G = np.stack([32.0 * (Wq[h] @ Wk[h].T) for h in range(H)])        # [H, D, D]
    F = np.stack([64.0 * (Wv[h] @ W1[h * D:(h + 1) * D]) for h in range(H)])
    wt = np.stack([32.0 * (Wk[h] @ bq[h]) for h in range(H)])          # [H, D]
    cvec = b1 + sum(bv[h] @ W1[h * D:(h + 1) * D] for h in range(H))   # [D]

    def pack_w(W, dtyp):  # [H, D, D] -> [H, c, P, kc, P] lhsT blocks
        return np.ascontiguousarray(
            W.reshape(H, ET, P, ET, P).transpose(0, 3, 2, 1, 4).astype(dtyp)
        )

    g8 = pack_w(G, F8)
    f8 = np.ascontiguousarray(F.astype(F8))
    wtr = np.ascontiguousarray(wt.reshape(H, ET, P).transpose(2, 0, 1).reshape(P, H * ET))
    w2b = np.ascontiguousarray(W2.astype(BF))
    buv = np.ascontiguousarray(b2.reshape(1, D).astype(BF))

    shared = {"g8": g8, "f8": f8, "wtr": wtr, "w2": w2b, "buv": buv}
    if not trivial:
        shared["gbe"] = np.ascontiguousarray(np.stack([g1, be1, g2, be2]))
    in_maps = []
    for core in range(8):
        b, half = core // 2, core % 2
        own = x[b, half * SI:(half + 1) * SI]
        other = x[b, (1 - half) * SI:(2 - half) * SI]
        xperm = np.concatenate([own, other], axis=0)
        m = dict(shared)
        m["x8"] = np.ascontiguousarray(xperm.T.astype(F8))
        m["xres"] = np.ascontiguousarray(own + cvec[None, :])
        in_maps.append(m)
    return trivial, in_maps


def kernel(**inputs):
    from concourse.bass_utils import run_bass_kernel_spmd

    trivial, in_maps = _prep_inputs(inputs)
    nc = _get_nc(trivial)
    res = run_bass_kernel_spmd(nc, in_maps, core_ids=list(range(8)))
    out = np.empty((4, S, D), np.float32)
    for core in range(8):
        b, half = core // 2, core % 2
        out[b, half * SI:(half + 1) * SI] = res.results[core]["out"]
    return out


# revision 36
# speedup vs baseline: 1.0490x; 1.0490x over previous
"""Trainium2 Bass kernel for nn_EncoderOnlyBlock (4-head full-dim encoder block).

Sharding: fully data-parallel, no collectives. 8 cores = (batch b, seq-half).
Each core computes its 1024 query tokens end-to-end for all 4 heads.

Algebraic folds (host, fp32):
  scores[i,j] = (x_i Wq + bq)·(x_j Wk + bk) / 32
              = (x_i G + w~)·x_j / 32  + terms constant over j (softmax-invariant)
    with G = Wq Wk^T, w~ = Wk bq.  K projection eliminated entirely.
  head_h @ W1_h = A_h @ (x F_h),  F_h = Wv_h W1_h;  bv/b1 fold into
    cvec = b1 + sum_h bv_h W1_h which is folded into xres on host.
  Softmax normalization deferred: proj_unnorm = sum_j exp(s) V', scaled by
    rec = 1/rowsum at PSUM->SBUF accumulation ([P,1] per-partition scalar).

fp8 (e4m3, DoubleRow perf mode = 2 K-tiles per pass) for all attention
matmuls: T' = x G'8, scores = T'8 @ x8, V' = x8 @ F'8, proj = expS8 @ V'8.
Host scales G'=32G, F'=64F keep fp8 operands in the normal range; the 32
folds into the exp scale (1/1024), the 64 into the u1 residual STT (1/64).
W2/LN tail stays bf16/f32 exactly as the bf16 baseline.

Per-core math:
  x_perm = [own-half tokens; other-half tokens]            (host permute)
  V'_h  = x_perm @ F'_h                     [sj, d]  fp8 out
  T'_h  = G'_h^T @ x_perm^T[:, :1024] + w~' [e, si]  fp8 out
  S     = T'^T x^T / 1024; E = exp(S) (fp8), r = rowsum(E) (f32 accum)
  proj += (E_h @ V'_h) * (1/r_h)            [si, d]  f32 accum in SBUF
  u1    = xres' + proj/64   (xres' = x_own + cvec, host)
  LN1 -> y; u2 = y + y@W2 + b2; LN2 -> out   (bf16 matmul, f32 LN)
"""

import numpy as np
import ml_dtypes

BF = ml_dtypes.bfloat16
F8 = ml_dtypes.float8_e4m3
P = 128
D = 1024
S = 2048
SI = 1024
H = 4
ET = D // P       # 8 e/d/f 128-blocks
SJT = S // P      # 16 sj 128-blocks
SIT = SI // P     # 8 si 128-blocks
EPS = 1e-5

_CACHE = {}


def _emit(nc, tc, A, trivial_gbe):
    """Emit the per-core program. A: dict name -> dram AP."""
    from contextlib import ExitStack

    import concourse.bass as bass
    import concourse.mybir as mybir
    from concourse.masks import make_identity

    f32 = mybir.dt.float32
    bf16 = mybir.dt.bfloat16
    f8 = mybir.dt.float8e4
    Act = mybir.ActivationFunctionType
    Alu = mybir.AluOpType
    DR = mybir.MatmulPerfMode.DoubleRow

    with ExitStack() as ctx:
        consts = ctx.enter_context(tc.tile_pool(name="consts", bufs=1))
        psA = ctx.enter_context(tc.tile_pool(name="psA", bufs=3, space="PSUM"))
        psB = ctx.enter_context(tc.tile_pool(name="psB", bufs=2, space="PSUM"))

        identb = consts.tile([P, P], bf16, tag="identb")
        make_identity(nc, identb[:])
        wtr_sb = consts.tile([P, H * ET], f32, tag="wtr")
        nc.sync.dma_start(out=wtr_sb[:], in_=A["wtr"][:])
        buv_sb = consts.tile([1, D], bf16, tag="buv")
        nc.sync.dma_start(out=buv_sb[:], in_=A["buv"][:])
        ones_sb = consts.tile([1, P], bf16, tag="ones")
        nc.vector.memset(ones_sb[:], 1.0)
        ones8 = consts.tile([P, 2, P], f8, tag="ones8")
        nc.vector.memset(ones8[:], 1.0)
        eps_sb = consts.tile([P, 1], f32, tag="eps")
        nc.vector.memset(eps_sb[:], EPS)

        head_ctx = ExitStack()
        xpool = head_ctx.enter_context(tc.tile_pool(name="xp", bufs=1))
        f_pool = head_ctx.enter_context(tc.tile_pool(name="fp8", bufs=2))
        g_pool = head_ctx.enter_context(tc.tile_pool(name="gp8", bufs=2))
        v_pool = head_ctx.enter_context(tc.tile_pool(name="vp", bufs=1))
        tt_pool = head_ctx.enter_context(tc.tile_pool(name="tt", bufs=2))
        e_pool = head_ctx.enter_context(tc.tile_pool(name="es", bufs=2))
        proj_pool = head_ctx.enter_context(tc.tile_pool(name="pj", bufs=1))
        red_pool = head_ctx.enter_context(tc.tile_pool(name="red", bufs=2))
        rec_pool = head_ctx.enter_context(tc.tile_pool(name="rec", bufs=2))

        x8_sb = xpool.tile([P, ET, S], f8, tag="x8")
        for hs in range(2):
            for c in range(ET):
                nc.sync.dma_start(
                    out=x8_sb[:, c, hs * SI:(hs + 1) * SI],
                    in_=A["x8"][c * P:(c + 1) * P, hs * SI:(hs + 1) * SI],
                )

        V8 = v_pool.tile([P, H, SJT, D], f8, tag="v8")
        proj_sb = proj_pool.tile([P, SIT, D], f32, tag="proj")

        # ---- V'_h = x @ F'_h for all 2048 rows, all heads, upfront
        for h in range(H):
            f8_sb = f_pool.tile([P, ET, D], f8, tag="f8")
            for kc in range(ET):
                nc.gpsimd.dma_start(
                    out=f8_sb[:, kc, :], in_=A["f8"][h, kc * P:(kc + 1) * P, :]
                )
            for j in range(SJT):
                ps = psA.tile([P, D], f32, tag="psA")
                for nb in range(2):
                    for kp in range(ET // 2):
                        nc.tensor.matmul(
                            ps[:, nb * 512:(nb + 1) * 512],
                            lhsT=x8_sb[:, 2 * kp:2 * kp + 2, j * P:(j + 1) * P],
                            rhs=f8_sb[:, 2 * kp:2 * kp + 2, nb * 512:(nb + 1) * 512],
                            start=(kp == 0), stop=(kp == ET // 2 - 1),
                            perf_mode=DR,
                        )
                nc.vector.tensor_copy(V8[:, h, j, :], ps[:])

        for h in range(H):
            # ---- T'^T = G'^T @ x^T[:, :1024] + w~' : [e, si]
            g8_sb = g_pool.tile([P, ET, ET, P], f8, tag="g8")
            for c in range(ET):
                nc.gpsimd.dma_start(out=g8_sb[:, c], in_=A["g8"][h, c])
            tt_sb = tt_pool.tile([P, ET, SI], f8, tag="tt")
            for c in range(ET):
                ps = psA.tile([P, D], f32, tag="psA")
                for nb in range(2):
                    for kp in range(ET // 2):
                        nc.tensor.matmul(
                            ps[:, nb * 512:(nb + 1) * 512],
                            lhsT=g8_sb[:, c, 2 * kp:2 * kp + 2, :],
                            rhs=x8_sb[:, 2 * kp:2 * kp + 2, nb * 512:(nb + 1) * 512],
                            start=(kp == 0), stop=(kp == ET // 2 - 1),
                            perf_mode=DR,
                        )
                nc.scalar.activation(
                    out=tt_sb[:, c, :], in_=ps[:], func=Act.Identity,
                    bias=wtr_sb[:, h * ET + c:h * ET + c + 1],
                )

            # ---- scores^T per key-block: S^T[j, i] = x_j . T'_i, exp'd straight
            # to fp8 (feeds proj as lhsT with no transposes). Row sums via tiny
            # ones-DoubleRow matmuls accumulated over all 16 key blocks.
            expS_sb = e_pool.tile([P, SJT, SI], f8, tag="expS")
            for jblk in range(SJT):
                ps = psA.tile([P, SI], f32, tag="psA")
                for nb in range(2):
                    for kp in range(ET // 2):
                        nc.tensor.matmul(
                            ps[:, nb * 512:(nb + 1) * 512],
                            lhsT=x8_sb[:, 2 * kp:2 * kp + 2, jblk * P:(jblk + 1) * P],
                            rhs=tt_sb[:, 2 * kp:2 * kp + 2, nb * 512:(nb + 1) * 512],
                            start=(kp == 0), stop=(kp == ET // 2 - 1),
                            perf_mode=DR,
                        )
                nc.scalar.activation(
                    out=expS_sb[:, jblk, :], in_=ps[:],
                    func=Act.Exp, scale=1.0 / 1024.0,
                )
            # rowsum broadcast to all 128 partitions via all-ones lhsT
            rps = psA.tile([P, SI], f32, tag="psA")
            for jp in range(SJT // 2):
                for nb in range(2):
                    nc.tensor.matmul(
                        rps[:, nb * 512:(nb + 1) * 512],
                        lhsT=ones8[:],
                        rhs=expS_sb[:, 2 * jp:2 * jp + 2, nb * 512:(nb + 1) * 512],
                        start=(jp == 0), stop=(jp == SJT // 2 - 1),
                        perf_mode=DR,
                    )
            # 1/rowsum, regrouped to [P si-in-tile, tile] via scatter DMA
            r_sb = red_pool.tile([P, SI], f32, tag="rrow")
            nc.vector.tensor_copy(r_sb[:], rps[:])
            nc.sync.dma_start(out=A["rsc"][h], in_=r_sb[0:1, :])
            rec_sb = rec_pool.tile([P, SIT], f32, tag="rec")
            nc.sync.dma_start(
                out=rec_sb[:], in_=A["rsc"][h].rearrange("(t p) -> p t", p=P)
            )
            nc.vector.reciprocal(rec_sb[:], rec_sb[:])

            # ---- proj += (expS_h @ V'_h) * rec_h
            for t in range(SIT):
                ps = psA.tile([P, D], f32, tag="psA")
                for nb in range(2):
                    for jp in range(SJT // 2):
                        nc.tensor.matmul(
                            ps[:, nb * 512:(nb + 1) * 512],
                            lhsT=expS_sb[:, 2 * jp:2 * jp + 2, t * P:(t + 1) * P],
                            rhs=V8[:, h, 2 * jp:2 * jp + 2, nb * 512:(nb + 1) * 512],
                            start=(jp == 0), stop=(jp == SJT // 2 - 1),
                            perf_mode=DR,
                        )
                if h == 0:
                    nc.vector.tensor_scalar_mul(
                        proj_sb[:, t, :], ps[:], rec_sb[:, t:t + 1]
                    )
                else:
                    nc.vector.scalar_tensor_tensor(
                        out=proj_sb[:, t, :], in0=ps[:], scalar=rec_sb[:, t:t + 1],
                        in1=proj_sb[:, t, :], op0=Alu.mult, op1=Alu.add,
                    )

        head_ctx.close()

        # ================= LN1 -> FFN2 -> LN2, fully per-si-tile =================
        with ExitStack() as lctx:
            lnp = lctx.enter_context(tc.tile_pool(name="lnp", bufs=1))
            xr_pool = lctx.enter_context(tc.tile_pool(name="xr", bufs=3))
            u_pool = lctx.enter_context(tc.tile_pool(name="up", bufs=3))
            sq_pool = lctx.enter_context(tc.tile_pool(name="sq", bufs=2))
            ybf_pool = lctx.enter_context(tc.tile_pool(name="ybf", bufs=2))
            yt_pool = lctx.enter_context(tc.tile_pool(name="yt", bufs=3))
            w2_pool = lctx.enter_context(tc.tile_pool(name="w2", bufs=1))
            st_pool = lctx.enter_context(tc.tile_pool(name="st", bufs=8))
            ot_pool = lctx.enter_context(tc.tile_pool(name="ot", bufs=3))

            if not trivial_gbe:
                gbe_sb = lnp.tile([P, 4, D], f32, tag="gbe")
                gbe_bc = bass.AP(
                    tensor=A["gbe"].tensor, offset=A["gbe"].offset,
                    ap=[[0, P], A["gbe"].ap[0], A["gbe"].ap[1]],
                )
                nc.gpsimd.dma_start(out=gbe_sb[:], in_=gbe_bc)
            y_sb = lnp.tile([P, SIT, D], f32, tag="y")

            xr_tiles = []
            for t in range(SIT):
                xr = xr_pool.tile([P, D], f32, tag="xr", name=f"xr{t}")
                nc.sync.dma_start(out=xr[:], in_=A["xres"][t * P:(t + 1) * P, :])
                xr_tiles.append(xr)

            w28_sb = w2_pool.tile([P, ET, D], bf16, tag="w2")
            for kc in range(ET):
                nc.sync.dma_start(
                    out=w28_sb[:, kc, :], in_=A["w2"][kc * P:(kc + 1) * P, :]
                )

            def ln_stats(src, rsum):
                """-> (mu, rstd) [P,1] tiles from src [P,D] + its row-sum."""
                sq = sq_pool.tile([P, D], f32, tag="sq")
                sumsq = st_pool.tile([P, 1], f32, tag="sumsq")
                nc.scalar.activation(out=sq[:], in_=src, func=Act.Square,
                                     accum_out=sumsq[:])
                mu = st_pool.tile([P, 1], f32, tag="mu")
                nc.scalar.mul(mu[:], rsum, 1.0 / D)
                # (rsum*mu - sumsq) = -D*var;  std = sqrt(-1/D * that + eps)
                nv = st_pool.tile([P, 1], f32, tag="nv")
                nc.vector.scalar_tensor_tensor(
                    out=nv[:], in0=rsum, scalar=mu[:], in1=sumsq[:],
                    op0=Alu.mult, op1=Alu.subtract,
                )
                rstd = st_pool.tile([P, 1], f32, tag="rstd")
                nc.scalar.activation(out=rstd[:], in_=nv[:], func=Act.Sqrt,
                                     scale=-1.0 / D, bias=eps_sb[:])
                nc.vector.reciprocal(rstd[:], rstd[:])
                return mu, rstd

            # Stage A (all tiles): u1 -> LN1 -> y -> yT. Emitting every tile's
            # A-stage before any B-stage keeps the in-order DVE queue from
            # serializing tile t+1's LN1 behind tile t's z-chain result.
            yt_tiles = []
            for t in range(SIT):
                # u1 = xres + proj/64, with row-sum accumulated in the same pass
                u1 = u_pool.tile([P, D], f32, tag="u")
                rs1 = st_pool.tile([P, 1], f32, tag="rs")
                nc.vector.scalar_tensor_tensor(
                    out=u1[:], in0=proj_sb[:, t, :], scalar=1.0 / 64.0,
                    in1=xr_tiles[t][:], op0=Alu.mult, op1=Alu.add,
                    accum_out=rs1[:],
                )
                mu1, rstd1 = ln_stats(u1[:], rs1[:])
                yt_t = y_sb[:, t, :]
                nc.vector.tensor_scalar(
                    yt_t, u1[:], scalar1=mu1[:], scalar2=rstd1[:],
                    op0=Alu.subtract, op1=Alu.mult,
                )
                if not trivial_gbe:
                    nc.gpsimd.tensor_mul(yt_t, yt_t, gbe_sb[:, 0, :])
                    nc.gpsimd.tensor_add(yt_t, yt_t, gbe_sb[:, 1, :])
                yb = ybf_pool.tile([P, D], bf16, tag="ybf")
                nc.scalar.copy(yb[:], yt_t)
                # transpose this tile's 8 f-blocks -> yT columns for its z-chain
                yt_tile = yt_pool.tile([P, ET, P], bf16, tag="yt", name=f"yt{t}")
                pb = psB.tile([P, 1024], bf16, tag="psB")
                for fb in range(ET):
                    nc.tensor.transpose(
                        pb[:, fb * P:(fb + 1) * P], yb[:, fb * P:(fb + 1) * P], identb[:]
                    )
                nc.vector.tensor_copy(
                    yt_tile[:], pb[:].rearrange("p (f c) -> p f c", c=P)
                )
                yt_tiles.append(yt_tile)

            # Stage B (all tiles): z-chain -> u2 -> LN2 -> out
            for t in range(SIT):
                ps = psA.tile([P, 1024], f32, tag="psA")
                for nb in range(2):
                    for kc in range(ET):
                        nc.tensor.matmul(
                            ps[:, nb * 512:(nb + 1) * 512],
                            lhsT=yt_tiles[t][:, kc, :],
                            rhs=w28_sb[:, kc, nb * 512:(nb + 1) * 512],
                            start=(kc == 0), stop=False,
                        )
                    nc.tensor.matmul(
                        ps[:, nb * 512:(nb + 1) * 512],
                        lhsT=ones_sb[:, :],
                        rhs=buv_sb[:, nb * 512:(nb + 1) * 512],
                        start=False, stop=True,
                    )
                u2 = u_pool.tile([P, D], f32, tag="u")
                rs2 = st_pool.tile([P, 1], f32, tag="rs")
                nc.vector.scalar_tensor_tensor(
                    out=u2[:], in0=ps[:], scalar=1.0,
                    in1=y_sb[:, t, :], op0=Alu.mult, op1=Alu.add,
                    accum_out=rs2[:],
                )
                mu2, rstd2 = ln_stats(u2[:], rs2[:])
                ot = ot_pool.tile([P, D], f32, tag="ot")
                nc.vector.tensor_scalar(
                    ot[:], u2[:], scalar1=mu2[:], scalar2=rstd2[:],
                    op0=Alu.subtract, op1=Alu.mult,
                )
                if not trivial_gbe:
                    nc.gpsimd.tensor_mul(ot[:], ot[:], gbe_sb[:, 2, :])
                    nc.gpsimd.tensor_add(ot[:], ot[:], gbe_sb[:, 3, :])
                eng = nc.sync if t % 2 == 0 else nc.gpsimd
                eng.dma_start(out=A["out"][t * P:(t + 1) * P, :], in_=ot[:])


def _build(trivial_gbe):
    import concourse.bass as bass
    import concourse.mybir as mybir
    import concourse.tile as tile
    from concourse import bacc

    f32 = mybir.dt.float32
    bf16 = mybir.dt.bfloat16
    f8 = mybir.dt.float8e4

    nc = bacc.Bacc("TRN2", target_bir_lowering=False, debug=False, num_devices=8)
    A = {}

    def din(name, shape, dt):
        A[name] = nc.dram_tensor(name, shape, dt, kind="ExternalInput").ap()

    din("x8", [D, S], f8)
    din("xres", [SI, D], f32)
    din("g8", [H, ET, P, ET, P], f8)
    din("f8", [H, D, D], f8)
    din("wtr", [P, H * ET], f32)
    din("w2", [D, D], bf16)
    din("buv", [1, D], bf16)
    if not trivial_gbe:
        din("gbe", [4, D], f32)
    A["rsc"] = nc.dram_tensor("rsc", [H, SI], f32, kind="Internal").ap()
    A["out"] = nc.dram_tensor("out", [SI, D], f32, kind="ExternalOutput").ap()

    with tile.TileContext(nc) as tc:
        _emit(nc, tc, A, trivial_gbe)
    nc.compile()
    return nc


def _get_nc(trivial_gbe=True):
    key = ("nc", trivial_gbe)
    if key not in _CACHE:
        _CACHE[key] = _build(trivial_gbe)
    return _CACHE[key]


def _prep_inputs(inputs):
    x = np.ascontiguousarray(inputs["embedding_matrix"], dtype=np.float32)
    Wq = np.asarray(inputs["Wq"], np.float32)
    bq = np.asarray(inputs["bq"], np.float32)
    Wv = np.asarray(inputs["Wv"], np.float32)
    bv = np.asarray(inputs["bv"], np.float32)
    Wk = np.asarray(inputs["Wk"], np.float32)
    W1 = np.asarray(inputs["W1"], np.float32)
    b1 = np.asarray(inputs["b1"], np.float32)
    W2 = np.asarray(inputs["W2"], np.float32)
    b2 = np.asarray(inputs["b2"], np.float32)
    g1 = np.asarray(inputs["g1"], np.float32)
    be1 = np.asarray(inputs["be1"], np.float32)
    g2 = np.asarray(inputs["g2"], np.float32)
    be2 = np.asarray(inputs["be2"], np.float32)

    trivial = (
        np.array_equal(g1, np.ones(D, np.float32))
        and np.array_equal(g2, np.ones(D, np.float32))
        and np.array_equal(be1, np.zeros(D, np.float32))
        and np.array_equal(be2, np.zeros(D, np.float32))
    )

    # host folds
    G = np.stack([32.0 * (Wq[h] @ Wk[h].T) for h in range(H)])        # [H, D, D]
    F = np.stack([64.0 * (Wv[h] @ W1[h * D:(h + 1) * D]) for h in range(H)])
    wt = np.stack([32.0 * (Wk[h] @ bq[h]) for h in range(H)])          # [H, D]
    cvec = b1 + sum(bv[h] @ W1[h * D:(h + 1) * D] for h in range(H))   # [D]

    def pack_w(W, dtyp):  # [H, D, D] -> [H, c, P, kc, P] lhsT blocks
        return np.ascontiguousarray(
            W.reshape(H, ET, P, ET, P).transpose(0, 3, 2, 1, 4).astype(dtyp)
        )

    g8 = pack_w(G, F8)
    f8 = np.ascontiguousarray(F.astype(F8))
    wtr = np.ascontiguousarray(wt.reshape(H, ET, P).transpose(2, 0, 1).reshape(P, H * ET))
    w2b = np.ascontiguousarray(W2.astype(BF))
    buv = np.ascontiguousarray(b2.reshape(1, D).astype(BF))

    shared = {"g8": g8, "f8": f8, "wtr": wtr, "w2": w2b, "buv": buv}
    if not trivial:
        shared["gbe"] = np.ascontiguousarray(np.stack([g1, be1, g2, be2]))
    in_maps = []
    for core in range(8):
        b, half = core // 2, core % 2
        own = x[b, half * SI:(half + 1) * SI]
        other = x[b, (1 - half) * SI:(2 - half) * SI]
        xperm = np.concatenate([own, other], axis=0)
        m = dict(shared)
        m["x8"] = np.ascontiguousarray(xperm.T.astype(F8))
        m["xres"] = np.ascontiguousarray(own + cvec[None, :])
        in_maps.append(m)
    return trivial, in_maps


def kernel(**inputs):
    from concourse.bass_utils import run_bass_kernel_spmd

    trivial, in_maps = _prep_inputs(inputs)
    nc = _get_nc(trivial)
    res = run_bass_kernel_spmd(nc, in_maps, core_ids=list(range(8)))
    out = np.empty((4, S, D), np.float32)
    for core in range(8):
        b, half = core // 2, core % 2
        out[b, half * SI:(half + 1) * SI] = res.results[core]["out"]
    return out
